# revision 1
# baseline (speedup 1.0000x reference)
"""Pyramid BiGRU encoder (4-layer, Keras reset_after GRU) on 8 TRN2 cores.

Sharding: 8 cores = 2 directions x 4 batch shards (8 sequences per core).
Every core runs an IDENTICAL bass program; direction-specific behavior is
data-driven (pre-reversed inputs/masks for bwd cores, host-provided gather
tables encode time reversal at pyramid boundaries). Cores exchange layer
outputs with pairwise AllGather collectives.

GRU step on device (z-column blocks of weights pre-negated on host so that
sigmoid of the raw PSUM value yields 1-z directly):
  psum = [ -pre_z | pre_r | h@rk_hh + b1_hh ]
  zc = sigmoid(psum_z); r = sigmoid(psum_r)
  w = zc * m_t;  u = r * psum_hh;  v = u + xz_hh;  hh = tanh(v)
  hn = h + w*(hh - h);  ys_t = hn * m_t
"""

from contextlib import ExitStack

import numpy as np

import concourse.bass as bass
import concourse.bacc as bacc
import concourse.mybir as mybir
import concourse.tile as tile
from concourse import bass_utils
from concourse.masks import make_identity

F32 = mybir.dt.float32
U32 = mybir.dt.uint32
AF = mybir.ActivationFunctionType

B, T1, F0, H = 32, 1024, 240, 512
G = 3 * H
NL = 4
NC = 8
Bs = B // (NC // 2)  # 8 sequences per core
NSL = 3


def build_encoder(T1=T1, Bs=Bs, F0=F0, n_layers=NL, n_cores=NC):
    nc = bacc.Bacc("TRN2", target_bir_lowering=False, debug=False,
                   num_devices=n_cores)
    Ts = [T1 >> l for l in range(n_layers)]
    Dins = [F0] + [4 * H] * (n_layers - 1)
    n_pairs = n_cores // 2
    groups = [[i, i + n_pairs] for i in range(n_pairs)]

    x0T = nc.dram_tensor("x0T", [F0, T1 * Bs], F32, kind="ExternalInput")
    wk = [nc.dram_tensor(f"wk{l}", [Dins[l], G], F32, kind="ExternalInput")
          for l in range(n_layers)]
    wr = [nc.dram_tensor(f"wr{l}", [H, G], F32, kind="ExternalInput")
          for l in range(n_layers)]
    bx = [nc.dram_tensor(f"bx{l}", [1, G], F32, kind="ExternalInput")
          for l in range(n_layers)]
    bh = [nc.dram_tensor(f"bh{l}", [1, H], F32, kind="ExternalInput")
          for l in range(n_layers)]
    mk = [nc.dram_tensor(f"mask{l}", [Bs, Ts[l]], F32, kind="ExternalInput")
          for l in range(n_layers)]
    tf, tb = [], []
    for l in range(n_layers - 1):
        cj = min(128, Ts[l])
        tf.append(nc.dram_tensor(f"tf{l}", [cj, Ts[l] // cj], U32,
                                 kind="ExternalInput"))
        tb.append(nc.dram_tensor(f"tb{l}", [cj, Ts[l] // cj], U32,
                                 kind="ExternalInput"))
    ys_out = nc.dram_tensor("ys_out", [Ts[-1] * Bs, H], F32,
                            kind="ExternalOutput")
    h_out = nc.dram_tensor("h_out", [Bs, H], F32, kind="ExternalOutput")

    with tile.TileContext(nc) as tc, ExitStack() as ctx:
        dram = ctx.enter_context(tc.tile_pool(name="dram", bufs=1, space="DRAM"))
        const = ctx.enter_context(tc.tile_pool(name="const", bufs=1))

        ident = const.tile([128, 128], F32)
        make_identity(nc, ident[:])
        ones = const.tile([1, 128], F32)
        nc.vector.memset(ones[:], 1.0)

        xz = [dram.tile([Ts[l] * Bs, G], F32, name=f"xz{l}")
              for l in range(n_layers)]
        ys = [dram.tile([Ts[l] * Bs, H], F32, name=f"ys{l}")
              for l in range(n_layers - 1)]
        R = [dram.tile([2 * Ts[l] * Bs, H], F32, name=f"R{l}")
             for l in range(n_layers - 1)]
        X = [dram.tile([Ts[l + 1] * Bs, 4 * H], F32, name=f"X{l + 1}")
             for l in range(n_layers - 1)]

        for l in range(n_layers):
            Tl, Din = Ts[l], Dins[l]
            # ---- xproj GEMM: xz[l] = X @ wk[l] + bx[l] ----
            with ExitStack() as pctx:
                wpool = pctx.enter_context(tc.tile_pool(name="wpool", bufs=1))
                lpool = pctx.enter_context(tc.tile_pool(name="lpool", bufs=2))
                epool = pctx.enter_context(tc.tile_pool(name="epool", bufs=3))
                gpsum = pctx.enter_context(
                    tc.tile_pool(name="gpsum", bufs=2, space="PSUM"))
                tpsum = pctx.enter_context(
                    tc.tile_pool(name="tpsum", bufs=2, space="PSUM"))

                kcs = [min(128, Din - i * 128)
                       for i in range((Din + 127) // 128)]
                nkc = len(kcs)
                wk_sb = wpool.tile([128, nkc, G], F32)
                for kc in range(nkc):
                    nc.sync.dma_start(
                        wk_sb[:kcs[kc], kc, :],
                        wk[l][kc * 128:kc * 128 + kcs[kc], :])
                bx_sb = wpool.tile([1, G], F32)
                nc.sync.dma_start(bx_sb[:], bx[l][:])

                rows = Tl * Bs
                nmt = rows // 128
                for mt in range(nmt):
                    lhsT = lpool.tile([128, nkc, 128], F32, name="lhsT")
                    if l == 0:
                        for kc in range(nkc):
                            nc.sync.dma_start(
                                lhsT[:kcs[kc], kc, :],
                                x0T[kc * 128:kc * 128 + kcs[kc],
                                    mt * 128:(mt + 1) * 128])
                    else:
                        xrow = lpool.tile([128, Din], F32, name="xrow")
                        nc.sync.dma_start(
                            xrow[:], X[l - 1][mt * 128:(mt + 1) * 128, :])
                        for kc in range(nkc):
                            pt = tpsum.tile([128, 128], F32, name="xt_ps")
                            nc.tensor.transpose(
                                pt[:], xrow[:, kc * 128:(kc + 1) * 128],
                                ident[:])
                            nc.vector.tensor_copy(lhsT[:, kc, :], pt[:])
                    for ns in range(NSL):
                        ps = gpsum.tile([128, 512], F32, name="xz_ps")
                        nsl = slice(ns * 512, (ns + 1) * 512)
                        nc.tensor.matmul(ps[:], ones[:1, :128], bx_sb[:, nsl],
                                         start=True, stop=False)
                        for kc in range(nkc):
                            nc.tensor.matmul(
                                ps[:], lhsT[:kcs[kc], kc, :],
                                wk_sb[:kcs[kc], kc, nsl],
                                start=False, stop=(kc == nkc - 1))
                        ev = epool.tile([128, 512], F32, name="xz_ev")
                        nc.scalar.copy(ev[:], ps[:])
                        nc.sync.dma_start(
                            xz[l][mt * 128:(mt + 1) * 128, nsl], ev[:])

            # ---- recurrent scan ----
            ys_dst = ys_out if l == n_layers - 1 else ys[l]
            with ExitStack() as sctx:
                spool = sctx.enter_context(tc.tile_pool(name="spool", bufs=1))
                hpool = sctx.enter_context(tc.tile_pool(name="hpool", bufs=2))
                xpool = sctx.enter_context(tc.tile_pool(name="xpool", bufs=6))
                gtmp = sctx.enter_context(tc.tile_pool(name="gtmp", bufs=2))
                ypool = sctx.enter_context(tc.tile_pool(name="ypool", bufs=4))
                spsum = sctx.enter_context(
                    tc.tile_pool(name="spsum", bufs=2, space="PSUM"))
                vpsum = sctx.enter_context(
                    tc.tile_pool(name="vpsum", bufs=2, space="PSUM"))

                rk_sb = spool.tile([128, 4, G], F32)
                nc.sync.dma_start(
                    rk_sb[:], wr[l][:].rearrange("(kc p) g -> p kc g", p=128))
                bh_sb = spool.tile([1, H], F32)
                nc.sync.dma_start(bh_sb[:], bh[l][:])
                m_sb = spool.tile([Bs, Ts[l]], F32)
                nc.sync.dma_start(m_sb[:], mk[l][:])

                h = hpool.tile([Bs, H], F32, name="h")
                hT = hpool.tile([128, 4 * Bs], F32, name="hT")
                nc.vector.memset(h[:], 0.0)
                nc.vector.memset(hT[:], 0.0)

                for t in range(Tl):
                    xzt = xpool.tile([Bs, G], F32, name="xzt")
                    nc.sync.dma_start(xzt[:], xz[l][t * Bs:(t + 1) * Bs, :])
                    ps = spsum.tile([Bs, G], F32, name="gates_ps")
                    for ns in range(2):
                        nsl = slice(ns * 512, (ns + 1) * 512)
                        nc.tensor.matmul(ps[:, nsl], ident[:Bs, :Bs],
                                         xzt[:, nsl], start=True, stop=False)
                        for kc in range(4):
                            nc.tensor.matmul(
                                ps[:, nsl], hT[:, kc * Bs:(kc + 1) * Bs],
                                rk_sb[:, kc, nsl], start=False,
                                stop=(kc == 3))
                    nsl = slice(1024, 1536)
                    nc.tensor.matmul(ps[:, nsl], ones[:1, :Bs], bh_sb[:],
                                     start=True, stop=False)
                    for kc in range(4):
                        nc.tensor.matmul(
                            ps[:, nsl], hT[:, kc * Bs:(kc + 1) * Bs],
                            rk_sb[:, kc, nsl], start=False, stop=(kc == 3))

                    zcr = gtmp.tile([Bs, 1024], F32, name="zcr")
                    nc.scalar.activation(zcr[:], ps[:, 0:1024], AF.Sigmoid)
                    w = gtmp.tile([Bs, H], F32, name="w")
                    nc.vector.tensor_scalar_mul(w[:], zcr[:, 0:H],
                                                m_sb[:, t:t + 1])
                    u = gtmp.tile([Bs, H], F32, name="u")
                    nc.vector.tensor_mul(u[:], zcr[:, H:1024],
                                         ps[:, 1024:1536])
                    v = gtmp.tile([Bs, H], F32, name="v")
                    nc.vector.tensor_add(v[:], u[:], xzt[:, 1024:1536])
                    hh = gtmp.tile([Bs, H], F32, name="hh")
                    nc.scalar.activation(hh[:], v[:], AF.Tanh)
                    d = gtmp.tile([Bs, H], F32, name="d")
                    nc.vector.tensor_sub(d[:], hh[:], h[:])
                    e = gtmp.tile([Bs, H], F32, name="e")
                    nc.vector.tensor_mul(e[:], w[:], d[:])
                    hn = hpool.tile([Bs, H], F32, name="h")
                    nc.vector.tensor_add(hn[:], h[:], e[:])
                    yst = ypool.tile([Bs, H], F32, name="yst")
                    nc.scalar.mul(yst[:], hn[:], m_sb[:, t:t + 1])
                    nc.sync.dma_start(ys_dst[t * Bs:(t + 1) * Bs, :], yst[:])
                    hTn = hpool.tile([128, 4 * Bs], F32, name="hT")
                    for half in range(2):
                        pt = vpsum.tile([128, 2 * Bs], F32, name="ht_ps")
                        for k2 in range(2):
                            kc = half * 2 + k2
                            nc.tensor.transpose(
                                pt[:, k2 * Bs:(k2 + 1) * Bs],
                                hn[:, kc * 128:(kc + 1) * 128],
                                ident[:Bs, :Bs])
                        if half == 0:
                            nc.scalar.copy(hTn[:, 0:2 * Bs], pt[:])
                        else:
                            nc.vector.tensor_copy(hTn[:, 2 * Bs:4 * Bs], pt[:])
                    h, hT = hn, hTn

                if l == n_layers - 1:
                    nc.sync.dma_start(h_out[:], h[:])

            # ---- boundary: pairwise exchange + stream gather ----
            if l < n_layers - 1:
                nc.gpsimd.collective_compute(
                    "AllGather", mybir.AluOpType.bypass,
                    replica_groups=groups,
                    ins=[ys[l][:]], outs=[R[l][:]])
                cj = min(128, Tl)
                nchunk = Tl // cj
                with ExitStack() as gctx:
                    gip = gctx.enter_context(tc.tile_pool(name="gip", bufs=1))
                    gsp = gctx.enter_context(tc.tile_pool(name="gsp", bufs=4))
                    tf_sb = gip.tile([cj, nchunk], U32, name="tf_sb")
                    tb_sb = gip.tile([cj, nchunk], U32, name="tb_sb")
                    nc.sync.dma_start(tf_sb[:], tf[l][:])
                    nc.sync.dma_start(tb_sb[:], tb[l][:])
                    Rv = R[l][:].rearrange("(s b) h -> s (b h)", b=Bs)
                    Xv = X[l][:].rearrange("(u b) (p s h) -> u p s b h",
                                           b=Bs, p=2, s=2)
                    for c in range(nchunk):
                        u0, nu = c * cj // 2, cj // 2
                        for si, tbl in ((0, tf_sb), (1, tb_sb)):
                            g = gsp.tile([cj, Bs * H], F32, name="gather")
                            nc.gpsimd.indirect_dma_start(
                                out=g[:], out_offset=None, in_=Rv,
                                in_offset=bass.IndirectOffsetOnAxis(
                                    ap=tbl[:, c:c + 1], axis=0))
                            for p in range(2):
                                nc.sync.dma_start(
                                    Xv[u0:u0 + nu, p, si, :, :],
                                    g[p * nu:(p + 1) * nu, :].rearrange(
                                        "u (b h) -> u b h", b=Bs))

    nc.compile()
    return nc


# ---------------- host side ----------------

def prep_core_inputs(x_full, mask_full, params, core, n_cores=NC):
    n_pairs = n_cores // 2
    is_bwd = core >= n_pairs
    shard = core % n_pairs
    bsl = slice(shard * Bs, (shard + 1) * Bs)
    d = 'b' if is_bwd else 'f'
    Ts = [T1 >> l for l in range(NL)]

    xc = x_full[bsl]
    mask = mask_full[bsl]
    if is_bwd:
        xc = xc[:, ::-1]
        mask = mask[:, ::-1]
    x0 = np.ascontiguousarray(
        xc.transpose(1, 0, 2).reshape(T1 * Bs, F0).T).astype(np.float32)

    im = {"x0T": x0}
    m = mask
    for l in range(NL):
        k = params[f'k{l + 1}{d}'].copy()
        r = params[f'r{l + 1}{d}'].copy()
        b = params[f'b{l + 1}{d}']
        k[:, :H] *= -1.0
        r[:, :H] *= -1.0
        bxv = np.concatenate([-(b[0, :H] + b[1, :H]),
                              b[0, H:2 * H] + b[1, H:2 * H],
                              b[0, 2 * H:]]).astype(np.float32)
        im[f"wk{l}"] = np.ascontiguousarray(k, np.float32)
        im[f"wr{l}"] = np.ascontiguousarray(r, np.float32)
        im[f"bx{l}"] = bxv[None, :]
        im[f"bh{l}"] = np.ascontiguousarray(b[1, 2 * H:], np.float32)[None, :]
        im[f"mask{l}"] = np.ascontiguousarray(m, np.float32)
        if l < NL - 1:
            Tl = Ts[l]
            j = np.arange(Tl)
            tmap = (Tl - 2 - (j & ~1) + (j & 1)) if is_bwd else j
            tfv = tmap
            tbv = 2 * Tl - 1 - tmap
            cjn = min(128, Tl)
            nu = cjn // 2
            nch = Tl // cjn

            def chunkify(v):
                v2 = np.empty_like(v)
                for c in range(nch):
                    for p in range(2):
                        v2[c * cjn + p * nu:(c * cjn + p * nu) + nu] = \
                            v[2 * c * nu + p: 2 * (c + 1) * nu + p: 2]
                return np.ascontiguousarray(
                    v2.reshape(nch, cjn).T.astype(np.uint32))

            im[f"tf{l}"] = chunkify(tfv)
            im[f"tb{l}"] = chunkify(tbv)
            m = m[:, 0::2] | m[:, 1::2]
    return im


_NC_CACHE = {}


def _get_nc():
    if "nc" not in _NC_CACHE:
        _NC_CACHE["nc"] = build_encoder()
    return _NC_CACHE["nc"]


def make_in_maps(inputs_arr, params):
    x = np.asarray(inputs_arr, np.float32)
    mask = ~np.all(x == 0.0, axis=-1)
    return [prep_core_inputs(x, mask, params, c) for c in range(NC)]


def assemble_outputs(results):
    n_pairs = NC // 2
    Tl = T1 >> (NL - 1)
    out = np.zeros((B, Tl, 2 * H), np.float32)
    hid = np.zeros((B, 2 * H), np.float32)
    for core, res in enumerate(results):
        is_bwd = core >= n_pairs
        shard = core % n_pairs
        bsl = slice(shard * Bs, (shard + 1) * Bs)
        ysc = res["ys_out"].reshape(Tl, Bs, H).transpose(1, 0, 2)
        if is_bwd:
            out[bsl, :, H:] = ysc[:, ::-1]
            hid[bsl, H:] = res["h_out"]
        else:
            out[bsl, :, :H] = ysc
            hid[bsl, :H] = res["h_out"]
    return out, hid


def kernel(inputs, initial_fwd_hidden, initial_bwd_hidden, **params):
    """Full-input entry point; returns (out, hidden_state) like reference."""
    nc = _get_nc()
    in_maps = make_in_maps(inputs, params)
    res = bass_utils.run_bass_kernel_spmd(nc, in_maps,
                                          core_ids=list(range(NC)))
    return assemble_outputs(res.results)
